# revision 37
# baseline (speedup 1.0000x reference)
"""Binarized conv2d kernel for Trainium2, SPMD over 8 NeuronCores.

Math (forward-value equivalent of the reference):
    real_w  = sum_k RV[k] * weights[k]          # [256,256,3,3], exact fp32 on DVE
    scale   = mean(|real_w|, axis=(1,2,3))      # per out-channel
    out     = conv2d(sign(x), sign(real_w), pad=1) * (scale * alpha)

sign(x) and sign(real_w) are {-1,0,+1} which are exact in fp8e4, so the conv
is computed with fp8 DoubleRow matmuls (exact integer accumulation in fp32
PSUM) and the per-channel scale*alpha is applied on PSUM evacuation.

Sharding: data-parallel over batch, 4 images per core; weights/RV/alpha
replicated. No collectives.
"""

import numpy as np
from contextlib import ExitStack

import concourse.bass as bass
import concourse.bacc as bacc
import concourse.tile as tile
from concourse import mybir
from concourse.bass_utils import run_bass_kernel_spmd
from concourse.masks import make_identity

# Problem shapes (hardcoded per contract)
B, C, H, W = 32, 256, 56, 56
K, KS = 4, 3
NCORES = 8
BL = B // NCORES            # images per core

PW = W + 2                  # padded width 58
PLANE = PW * PW             # 3364
PL = 3376                   # plane stride (>= 1+PLANE+1, multiple of 16)
GO = 1                      # guard offset: plane data starts at elem 1
RPC = 8                     # rows per chunk
CHUNK = RPC * PW            # 464 elems per matmul (one PSUM bank)
NCHUNK = H // RPC           # 7 chunks: psum tile A gets 4, tile B gets 3
PT_CHUNKS = (4, 3)
CIH = C // 128              # 2 ci halves
COH = C // 128              # 2 co halves
TAPS = KS * KS              # 9

F32 = mybir.dt.float32
FP8 = mybir.dt.float8e4
BF16 = mybir.dt.bfloat16

USE_DR = True               # fp8 DoubleRow (2x matmul) vs bf16

_cache = {}


def _build():
    act_dt = FP8 if USE_DR else BF16
    nc = bacc.Bacc("TRN2", target_bir_lowering=False, debug=False,
                   num_devices=NCORES)
    x_d = nc.dram_tensor("x", [BL, C, H, W], F32, kind="ExternalInput")
    w_d = nc.dram_tensor("weights", [K, C, C, KS, KS], F32, kind="ExternalInput")
    rv_d = nc.dram_tensor("RV", [K + 1], F32, kind="ExternalInput")
    al_d = nc.dram_tensor("alpha", [C, 1, 1], F32, kind="ExternalInput")
    o_d = nc.dram_tensor("out", [BL, C, H, W], F32, kind="ExternalOutput")

    with tile.TileContext(nc) as tc, ExitStack() as ctx:
        consts = ctx.enter_context(tc.tile_pool(name="consts", bufs=1))
        wstage = ctx.enter_context(tc.tile_pool(name="wstage", bufs=16))
        wwork = ctx.enter_context(tc.tile_pool(name="wwork", bufs=2))
        xin = ctx.enter_context(tc.tile_pool(name="xin", bufs=2))
        xpads = ctx.enter_context(tc.tile_pool(name="xpads", bufs=1))
        outp = ctx.enter_context(tc.tile_pool(name="outp", bufs=2))

        # --- tiny constant loads on the ACT HWDGE ring (keeps the sync
        # ring free for the big weight DMAs) -------------------------------
        rv = consts.tile([128, K], F32, tag="rv")
        rv_src = bass.AP(tensor=rv_d.ap().tensor, offset=0,
                         ap=[[0, 128], [1, K]])
        nc.scalar.dma_start(out=rv, in_=rv_src)
        alpha_sb = []
        for h in range(COH):
            t = consts.tile([128, 1], F32, tag=f"alpha{h}")
            nc.scalar.dma_start(out=t,
                                in_=al_d.ap()[h * 128:(h + 1) * 128, 0, :])
            alpha_sb.append(t)

        # Padded planes: zero only the pad borders on DVE (tiny strided
        # memsets — the interior is fully overwritten by sign(x) each image
        # and pads are never written again). GpSimd keeps only the identity.
        xpad = []
        for i in range(2):
            t = xpads.tile([128, CIH, PL], act_dt, tag=f"xpad{i}",
                           name=f"xpad{i}")
            for s in range(CIH):
                pl = t[:, s, :]
                # guard + top row + (1,0)
                nc.vector.memset(pl[:, 0:GO + PW + 1], 0.0)
                # (y,0) and (y,57) for y=1..56
                nc.vector.memset(
                    pl[:, GO + PW:GO + PW + H * PW].rearrange(
                        "p (r c) -> p r c", c=PW)[:, :, 0:1], 0.0)
                nc.vector.memset(
                    pl[:, GO + PW + PW - 1:GO + PW + PW - 1 + H * PW].rearrange(
                        "p (r c) -> p r c", c=PW)[:, :, 0:1], 0.0)
                # bottom row + trailing guard/pad
                nc.vector.memset(pl[:, GO + (PW - 1) * PW:PL], 0.0)
            xpad.append(t)
        ident = consts.tile([128, 128], act_dt, tag="ident")
        make_identity(nc, ident)

        wT = consts.tile([128, TAPS, COH, CIH, 128], act_dt, tag="wT")
        scale_alpha = [consts.tile([128, 1], F32, tag=f"sa{h}", name=f"sa{h}")
                       for h in range(COH)]

        # --- weight prep for one co-half: DMA, mix, scale, sign ------------
        # ci-split (HCI columns at a time) so the mix/sign tail trails the
        # weight DMA by one sub-pass instead of the whole 4.7MB.
        HCI = C // CIH * TAPS  # 1152 columns per ci-half
        def prep_half(h):
            # (TensorScalarPtr is DVE-only in walrus codegen — Pool rejects)
            mixeng = nc.vector
            wmix = wwork.tile([128, C * TAPS], F32, tag="wmix", name="wmix")
            ws = wwork.tile([128, C * TAPS], act_dt, tag=f"wsign{h}", bufs=1,
                            name=f"wsign{h}")
            wks = []
            for ci in range(CIH):
                for k in range(K):
                    wk = wstage.tile([128, HCI], F32, tag="wsb", name="wk")
                    wks.append(wk)
                    nc.sync.dma_start(
                        out=wk,
                        in_=w_d.ap()[k, h * 128:(h + 1) * 128,
                                     ci * (C // CIH):(ci + 1) * (C // CIH)]
                        .rearrange("p c a b -> p (c a b)"))
                    dst = wmix[:, ci * HCI:(ci + 1) * HCI]
                    mixeng.scalar_tensor_tensor(
                        dst, wk, rv[:, k:k + 1], wk if k == 0 else dst,
                        mybir.AluOpType.mult,
                        mybir.AluOpType.bypass if k == 0 else
                        mybir.AluOpType.add)
                nc.scalar.sign(ws[:, ci * HCI:(ci + 1) * HCI],
                               wmix[:, ci * HCI:(ci + 1) * HCI])
            return ws, wmix, wks

        # |real_w| row-sums + scale*alpha combine, on DVE (fills DMA-gated
        # bubbles between mix passes)
        def reduce_half(h, wmix):
            absum = consts.tile([128, 1], F32, tag=f"ab{h}", name=f"ab{h}")
            nc.vector.tensor_reduce(absum, wmix, mybir.AxisListType.X,
                                    mybir.AluOpType.add,
                                    apply_absolute_value=True)
            nc.vector.scalar_tensor_tensor(
                scale_alpha[h], absum, 1.0 / (C * TAPS), alpha_sb[h],
                mybir.AluOpType.mult, mybir.AluOpType.mult)

        # --- transpose one co-half's sign-weights into wT -------------------
        # The 18 [128,128] transposes are staged across three PSUM regions
        # (psB: taps 0-5, tps: taps 6-7, psA: tap 8) so the matmuls run
        # back-to-back with only 3 ACT copies and no copy-WAR stalls.
        def transpose_half(h, wsgn, cpsum):
            wsv = wsgn.rearrange("p (ci t) -> p ci t", t=TAPS)
            if h == 0:
                # ps0 first (frees conv00's first psum tile early); psB copy
                # split in two so conv00's tap-0 weights land sooner
                stages = [("ps0", 4 * 512, 8, 9, 1), ("ps1", 3 * 512, 0, 6, 2),
                          ("tps", 512, 6, 8, 1)]
            else:
                # ps1 last: it must wait for the previous conv's ptile-B
                # evacuation, which lands latest
                stages = [("ps0", 4 * 512, 0, 2, 1), ("tps", 512, 2, 4, 1),
                          ("ps1", 3 * 512, 4, 9, 2)]
            for tag, width, ta, tb, ncopy in stages:
                tp = cpsum.tile([128, width], F32, tag=tag, bufs=1,
                                name=f"t{tag}")
                for i, (tap, ci) in enumerate(
                        [(t, c) for t in range(ta, tb) for c in range(CIH)]):
                    nc.tensor.matmul(
                        tp[:, i * 128:(i + 1) * 128],
                        wsv[:, ci * 128:(ci + 1) * 128, tap], ident,
                        start=True, stop=True)
                nt = tb - ta
                for ic in range(ncopy):
                    ca = ta + ic * nt // ncopy
                    cb = ta + (ic + 1) * nt // ncopy
                    o0 = (ca - ta) * CIH * 128
                    nc.scalar.copy(
                        wT[:, ca:cb, h, :, :],
                        tp[:, o0:o0 + (cb - ca) * CIH * 128].rearrange(
                            "p (t ci co) -> p t ci co", t=cb - ca, co=128))

        # --- load + sign one image into its padded plane --------------------
        # DMA rides the sync ring (explicit bandwidth ordering vs weights);
        # the ACT sign is emitted separately so ACT priority is controlled.
        def load(b):
            tiles = []
            for s in range(CIH):
                xs = xin.tile([128, H * W], F32, tag="xsb", name="xsb")
                nc.sync.dma_start(
                    out=xs, in_=x_d.ap()[b, s * 128:(s + 1) * 128].rearrange(
                        "p a b -> p (a b)"))
                tiles.append(xs)
            return tiles

        def sign(b, tiles):
            xp = xpad[b % 2]
            for s in range(CIH):
                dst = xp[:, s, GO:GO + PLANE].rearrange(
                    "p (y x) -> p y x", x=PW)[:, 1:57, 1:57]
                nc.scalar.sign(dst, tiles[s].rearrange("p (y x) -> p y x", x=W))

        # --- conv for one (image, co-half) ---------------------------------
        def conv(b, h, cpsum):
            xp = xpad[b % 2]
            osb = outp.tile([128, H * W], F32, tag="osb", name="osb")
            c0 = 0
            for t, nch in enumerate(PT_CHUNKS):
                ps = cpsum.tile([128, nch * 512], F32, tag=f"ps{t}", bufs=1,
                                name=f"ps{t}")
                for tap in range(TAPS):
                    dy, dx = tap // KS - 1, tap % KS - 1
                    lhsT = wT[:, tap, h, :, :]
                    for j in range(nch):
                        c = c0 + j
                        off = GO + (1 + RPC * c + dy) * PW + dx
                        o = ps[:, j * 512:j * 512 + CHUNK]
                        if USE_DR:
                            nc.tensor.matmul(
                                o, lhsT, xp[:, :, off:off + CHUNK],
                                start=(tap == 0), stop=(tap == TAPS - 1),
                                perf_mode=mybir.MatmulPerfMode.DoubleRow)
                        else:
                            for s in range(CIH):
                                nc.tensor.matmul(
                                    o, wT[:, tap, h, s, :],
                                    xp[:, s, off:off + CHUNK],
                                    start=(tap == 0 and s == 0),
                                    stop=(tap == TAPS - 1 and s == CIH - 1))
                src = ps.rearrange("p (c e) -> p c e", e=512)[
                    :, 0:nch, 0:CHUNK].rearrange(
                    "p c (r x) -> p c r x", x=PW)[:, :, :, 1:57]
                dst = osb.rearrange("p (y x) -> p y x", x=W)[
                    :, c0 * RPC:(c0 + nch) * RPC, :].rearrange(
                    "p (c r) x -> p c r x", r=RPC)
                # evac split: ptile A on ACT, ptile B on DVE (DVE is idle in
                # steady state; during startup the B evac lands after the
                # weight mix is done, so the mix keeps DVE priority)
                if t == 0:
                    nc.scalar.activation(dst, src,
                                         mybir.ActivationFunctionType.Copy,
                                         bias=0.0, scale=scale_alpha[h])
                else:
                    nc.vector.tensor_scalar(dst, src, scale_alpha[h], None,
                                            mybir.AluOpType.mult)
                # per-ptile output DMA on the ACT ring (keeps the sync ring
                # free for input DMAs and avoids head-of-line blocking)
                nc.scalar.dma_start(
                    out=o_d.ap()[b, h * 128:(h + 1) * 128,
                                 c0 * RPC:(c0 + nch) * RPC, :].rearrange(
                        "p a b -> p (a b)"),
                    in_=osb[:, c0 * RPC * W:(c0 + nch) * RPC * W])
                c0 += nch

        # --- schedule --------------------------------------------------------
        # x DMAs ride the ACT ring (issue immediately); sign-x sits after
        # sign-w in program order so the weight chain keeps ACT priority.
        with tc.tile_pool(name="cpsum", bufs=1, space="PSUM") as cpsum:
            xt0 = load(0)              # sync-ring order: x0, w-h0, w-h1, x1..
            sign(0, xt0)
            ws0, wm0, wks0 = prep_half(0)
            # HAM warmup: three fp32 matmuls (~4.6us of PE busy) gated on a
            # mid-stream weight tile so the clock gate opens before the real
            # transposes/convs arrive. Results land in tps and are discarded.
            for i in range(3):
                wtp = cpsum.tile([128, 512], F32, tag="tps", bufs=1,
                                 name="warm")
                nc.tensor.matmul(wtp[:, 0:464], wks0[5][:, 0:128],
                                 wks0[5][:, 0:464], start=True, stop=True)
            transpose_half(0, ws0, cpsum)
            reduce_half(0, wm0)
            ws1, wm1, _ = prep_half(1)
            conv(0, 0, cpsum)
            transpose_half(1, ws1, cpsum)
            reduce_half(1, wm1)
            xt1 = load(1)
            sign(1, xt1)
            conv(0, 1, cpsum)
            for b in range(1, BL):
                if b + 1 < BL:
                    xt = load(b + 1)   # prefetch ahead of this image's evacs
                    sign(b + 1, xt)
                conv(b, 0, cpsum)
                conv(b, 1, cpsum)
    nc.compile()
    return nc


def _get_nc():
    if "nc" not in _cache:
        _cache["nc"] = _build()
    return _cache["nc"]


def run(inputs, trace=False):
    nc = _get_nc()
    x = np.ascontiguousarray(inputs["x"], dtype=np.float32)
    in_maps = [
        {
            "x": x[c * BL:(c + 1) * BL],
            "weights": np.ascontiguousarray(inputs["weights"], np.float32),
            "RV": np.ascontiguousarray(inputs["RV"], np.float32),
            "alpha": np.ascontiguousarray(inputs["alpha"], np.float32),
        }
        for c in range(NCORES)
    ]
    res = run_bass_kernel_spmd(nc, in_maps, core_ids=list(range(NCORES)),
                               trace=trace)
    out = np.concatenate([r["out"] for r in res.results], axis=0)
    return out, res


def kernel(**inputs) -> np.ndarray:
    out, _ = run(inputs, trace=False)
    return out


# revision 38
# speedup vs baseline: 1.0560x; 1.0560x over previous
"""Binarized conv2d kernel for Trainium2, SPMD over 8 NeuronCores.

Math (forward-value equivalent of the reference):
    real_w  = sum_k RV[k] * weights[k]          # [256,256,3,3], exact fp32 on DVE
    scale   = mean(|real_w|, axis=(1,2,3))      # per out-channel
    out     = conv2d(sign(x), sign(real_w), pad=1) * (scale * alpha)

sign(x) and sign(real_w) are {-1,0,+1} which are exact in fp8e4, so the conv
is computed with fp8 DoubleRow matmuls (exact integer accumulation in fp32
PSUM) and the per-channel scale*alpha is applied on PSUM evacuation.

Sharding: data-parallel over batch, 4 images per core; weights/RV/alpha
replicated. No collectives.
"""

import numpy as np
from contextlib import ExitStack

import concourse.bass as bass
import concourse.bacc as bacc
import concourse.tile as tile
from concourse import mybir
from concourse.bass_utils import run_bass_kernel_spmd
from concourse.masks import make_identity

# Problem shapes (hardcoded per contract)
B, C, H, W = 32, 256, 56, 56
K, KS = 4, 3
NCORES = 8
BL = B // NCORES            # images per core

PW = W + 2                  # padded width 58
PLANE = PW * PW             # 3364
PL = 3376                   # plane stride (>= 1+PLANE+1, multiple of 16)
GO = 1                      # guard offset: plane data starts at elem 1
RPC = 8                     # rows per chunk
CHUNK = RPC * PW            # 464 elems per matmul (one PSUM bank)
NCHUNK = H // RPC           # 7 chunks: psum tile A gets 4, tile B gets 3
PT_CHUNKS = (4, 3)
CIH = C // 128              # 2 ci halves
COH = C // 128              # 2 co halves
TAPS = KS * KS              # 9

F32 = mybir.dt.float32
FP8 = mybir.dt.float8e4
BF16 = mybir.dt.bfloat16

USE_DR = True               # fp8 DoubleRow (2x matmul) vs bf16

_cache = {}


def _build():
    act_dt = FP8 if USE_DR else BF16
    nc = bacc.Bacc("TRN2", target_bir_lowering=False, debug=False,
                   num_devices=NCORES)
    x_d = nc.dram_tensor("x", [BL, C, H, W], F32, kind="ExternalInput")
    w_d = nc.dram_tensor("weights", [K, C, C, KS, KS], F32, kind="ExternalInput")
    rv_d = nc.dram_tensor("RV", [K + 1], F32, kind="ExternalInput")
    al_d = nc.dram_tensor("alpha", [C, 1, 1], F32, kind="ExternalInput")
    o_d = nc.dram_tensor("out", [BL, C, H, W], F32, kind="ExternalOutput")

    with tile.TileContext(nc) as tc, ExitStack() as ctx:
        consts = ctx.enter_context(tc.tile_pool(name="consts", bufs=1))
        wstage = ctx.enter_context(tc.tile_pool(name="wstage", bufs=16))
        wwork = ctx.enter_context(tc.tile_pool(name="wwork", bufs=2))
        xin = ctx.enter_context(tc.tile_pool(name="xin", bufs=2))
        xpads = ctx.enter_context(tc.tile_pool(name="xpads", bufs=1))
        outp = ctx.enter_context(tc.tile_pool(name="outp", bufs=2))

        # --- tiny constant loads on the ACT HWDGE ring (keeps the sync
        # ring free for the big weight DMAs) -------------------------------
        rv = consts.tile([128, K], F32, tag="rv")
        rv_src = bass.AP(tensor=rv_d.ap().tensor, offset=0,
                         ap=[[0, 128], [1, K]])
        nc.scalar.dma_start(out=rv, in_=rv_src)
        alpha_sb = []
        for h in range(COH):
            t = consts.tile([128, 1], F32, tag=f"alpha{h}")
            nc.scalar.dma_start(out=t,
                                in_=al_d.ap()[h * 128:(h + 1) * 128, 0, :])
            alpha_sb.append(t)

        # Padded planes: zero only the pad borders on DVE (tiny strided
        # memsets — the interior is fully overwritten by sign(x) each image
        # and pads are never written again). GpSimd keeps only the identity.
        xpad = []
        for i in range(2):
            t = xpads.tile([128, CIH, PL], act_dt, tag=f"xpad{i}",
                           name=f"xpad{i}")
            for s in range(CIH):
                pl = t[:, s, :]
                # guard + top row + (1,0)
                nc.vector.memset(pl[:, 0:GO + PW + 1], 0.0)
                # (y,0) and (y,57) for y=1..56
                nc.vector.memset(
                    pl[:, GO + PW:GO + PW + H * PW].rearrange(
                        "p (r c) -> p r c", c=PW)[:, :, 0:1], 0.0)
                nc.vector.memset(
                    pl[:, GO + PW + PW - 1:GO + PW + PW - 1 + H * PW].rearrange(
                        "p (r c) -> p r c", c=PW)[:, :, 0:1], 0.0)
                # bottom row + trailing guard/pad
                nc.vector.memset(pl[:, GO + (PW - 1) * PW:PL], 0.0)
            xpad.append(t)
        ident = consts.tile([128, 128], act_dt, tag="ident")
        make_identity(nc, ident)

        wT = consts.tile([128, TAPS, COH, CIH, 128], act_dt, tag="wT")
        scale_alpha = [consts.tile([128, 1], F32, tag=f"sa{h}", name=f"sa{h}")
                       for h in range(COH)]

        # --- weight prep for one co-half: DMA, mix, scale, sign ------------
        # ci-split (HCI columns at a time) so the mix/sign tail trails the
        # weight DMA by one sub-pass instead of the whole 4.7MB.
        HCI = C // CIH * TAPS  # 1152 columns per ci-half
        def prep_half(h):
            # (TensorScalarPtr is DVE-only in walrus codegen — Pool rejects)
            mixeng = nc.vector
            wmix = wwork.tile([128, C * TAPS], F32, tag="wmix", name="wmix")
            ws = wwork.tile([128, C * TAPS], act_dt, tag=f"wsign{h}", bufs=1,
                            name=f"wsign{h}")
            wks = []
            for ci in range(CIH):
                for k in range(K):
                    wk = wstage.tile([128, HCI], F32, tag="wsb", name="wk")
                    wks.append(wk)
                    nc.sync.dma_start(
                        out=wk,
                        in_=w_d.ap()[k, h * 128:(h + 1) * 128,
                                     ci * (C // CIH):(ci + 1) * (C // CIH)]
                        .rearrange("p c a b -> p (c a b)"))
                    dst = wmix[:, ci * HCI:(ci + 1) * HCI]
                    mixeng.scalar_tensor_tensor(
                        dst, wk, rv[:, k:k + 1], wk if k == 0 else dst,
                        mybir.AluOpType.mult,
                        mybir.AluOpType.bypass if k == 0 else
                        mybir.AluOpType.add)
                nc.scalar.sign(ws[:, ci * HCI:(ci + 1) * HCI],
                               wmix[:, ci * HCI:(ci + 1) * HCI])
            return ws, wmix, wks

        # |real_w| row-sums + scale*alpha combine, on DVE (fills DMA-gated
        # bubbles between mix passes)
        def reduce_half(h, wmix):
            absum = consts.tile([128, 1], F32, tag=f"ab{h}", name=f"ab{h}")
            nc.vector.tensor_reduce(absum, wmix, mybir.AxisListType.X,
                                    mybir.AluOpType.add,
                                    apply_absolute_value=True)
            nc.vector.scalar_tensor_tensor(
                scale_alpha[h], absum, 1.0 / (C * TAPS), alpha_sb[h],
                mybir.AluOpType.mult, mybir.AluOpType.mult)

        # --- transpose one co-half's sign-weights into wT -------------------
        # The 18 [128,128] transposes are staged across three PSUM regions
        # (psB: taps 0-5, tps: taps 6-7, psA: tap 8) so the matmuls run
        # back-to-back with only 3 ACT copies and no copy-WAR stalls.
        def transpose_half(h, wsgn, cpsum):
            wsv = wsgn.rearrange("p (ci t) -> p ci t", t=TAPS)
            if h == 0:
                # ps0 first (frees conv00's first psum tile early); psB copy
                # split in two so conv00's tap-0 weights land sooner
                stages = [("ps0", 4 * 512, 8, 9, 1), ("ps1", 3 * 512, 0, 6, 2),
                          ("tps", 512, 6, 8, 1)]
            else:
                # ps1 last: it must wait for the previous conv's ptile-B
                # evacuation, which lands latest
                stages = [("ps0", 4 * 512, 0, 2, 1), ("tps", 512, 2, 4, 1),
                          ("ps1", 3 * 512, 4, 9, 2)]
            for tag, width, ta, tb, ncopy in stages:
                tp = cpsum.tile([128, width], F32, tag=tag, bufs=1,
                                name=f"t{tag}")
                for i, (tap, ci) in enumerate(
                        [(t, c) for t in range(ta, tb) for c in range(CIH)]):
                    nc.tensor.matmul(
                        tp[:, i * 128:(i + 1) * 128],
                        wsv[:, ci * 128:(ci + 1) * 128, tap], ident,
                        start=True, stop=True)
                nt = tb - ta
                for ic in range(ncopy):
                    ca = ta + ic * nt // ncopy
                    cb = ta + (ic + 1) * nt // ncopy
                    o0 = (ca - ta) * CIH * 128
                    nc.scalar.copy(
                        wT[:, ca:cb, h, :, :],
                        tp[:, o0:o0 + (cb - ca) * CIH * 128].rearrange(
                            "p (t ci co) -> p t ci co", t=cb - ca, co=128))

        # --- load + sign one image into its padded plane --------------------
        # DMA rides the sync ring (explicit bandwidth ordering vs weights);
        # the ACT sign is emitted separately so ACT priority is controlled.
        def load(b):
            tiles = []
            for s in range(CIH):
                xs = xin.tile([128, H * W], F32, tag="xsb", name="xsb")
                nc.sync.dma_start(
                    out=xs, in_=x_d.ap()[b, s * 128:(s + 1) * 128].rearrange(
                        "p a b -> p (a b)"))
                tiles.append(xs)
            return tiles

        def sign(b, tiles):
            xp = xpad[b % 2]
            for s in range(CIH):
                dst = xp[:, s, GO:GO + PLANE].rearrange(
                    "p (y x) -> p y x", x=PW)[:, 1:57, 1:57]
                nc.scalar.sign(dst, tiles[s].rearrange("p (y x) -> p y x", x=W))

        # --- conv for one (image, co-half) ---------------------------------
        def conv(b, h, cpsum):
            xp = xpad[b % 2]
            osb = outp.tile([128, H * W], F32, tag="osb", name="osb")
            c0 = 0
            for t, nch in enumerate(PT_CHUNKS):
                ps = cpsum.tile([128, nch * 512], F32, tag=f"ps{t}", bufs=1,
                                name=f"ps{t}")
                for tap in range(TAPS):
                    dy, dx = tap // KS - 1, tap % KS - 1
                    lhsT = wT[:, tap, h, :, :]
                    for j in range(nch):
                        c = c0 + j
                        off = GO + (1 + RPC * c + dy) * PW + dx
                        o = ps[:, j * 512:j * 512 + CHUNK]
                        if USE_DR:
                            nc.tensor.matmul(
                                o, lhsT, xp[:, :, off:off + CHUNK],
                                start=(tap == 0), stop=(tap == TAPS - 1),
                                perf_mode=mybir.MatmulPerfMode.DoubleRow)
                        else:
                            for s in range(CIH):
                                nc.tensor.matmul(
                                    o, wT[:, tap, h, s, :],
                                    xp[:, s, off:off + CHUNK],
                                    start=(tap == 0 and s == 0),
                                    stop=(tap == TAPS - 1 and s == CIH - 1))
                src = ps.rearrange("p (c e) -> p c e", e=512)[
                    :, 0:nch, 0:CHUNK].rearrange(
                    "p c (r x) -> p c r x", x=PW)[:, :, :, 1:57]
                dst = osb.rearrange("p (y x) -> p y x", x=W)[
                    :, c0 * RPC:(c0 + nch) * RPC, :].rearrange(
                    "p (c r) x -> p c r x", r=RPC)
                # all PSUM evacuation on ACT (moving any of it to DVE lets
                # the static scheduler interleave it with the weight mix,
                # which measurably regresses)
                nc.scalar.activation(dst, src,
                                     mybir.ActivationFunctionType.Copy,
                                     bias=0.0, scale=scale_alpha[h])
                # per-ptile output DMA on the ACT ring (keeps the sync ring
                # free for input DMAs and avoids head-of-line blocking)
                nc.scalar.dma_start(
                    out=o_d.ap()[b, h * 128:(h + 1) * 128,
                                 c0 * RPC:(c0 + nch) * RPC, :].rearrange(
                        "p a b -> p (a b)"),
                    in_=osb[:, c0 * RPC * W:(c0 + nch) * RPC * W])
                c0 += nch

        # --- schedule --------------------------------------------------------
        # x DMAs ride the ACT ring (issue immediately); sign-x sits after
        # sign-w in program order so the weight chain keeps ACT priority.
        with tc.tile_pool(name="cpsum", bufs=1, space="PSUM") as cpsum:
            xt0 = load(0)              # sync-ring order: x0, w-h0, w-h1, x1..
            sign(0, xt0)
            ws0, wm0, wks0 = prep_half(0)
            # HAM warmup: three fp32 matmuls (~4.6us of PE busy) gated on a
            # mid-stream weight tile so the clock gate opens before the real
            # transposes/convs arrive. Results land in tps and are discarded.
            for i in range(3):
                wtp = cpsum.tile([128, 512], F32, tag="tps", bufs=1,
                                 name="warm")
                nc.tensor.matmul(wtp[:, 0:464], wks0[5][:, 0:128],
                                 wks0[5][:, 0:464], start=True, stop=True)
            transpose_half(0, ws0, cpsum)
            reduce_half(0, wm0)
            ws1, wm1, _ = prep_half(1)
            conv(0, 0, cpsum)
            transpose_half(1, ws1, cpsum)
            reduce_half(1, wm1)
            xt1 = load(1)
            sign(1, xt1)
            conv(0, 1, cpsum)
            for b in range(1, BL):
                if b + 1 < BL:
                    xt = load(b + 1)   # prefetch ahead of this image's evacs
                    sign(b + 1, xt)
                conv(b, 0, cpsum)
                conv(b, 1, cpsum)
    nc.compile()
    return nc


def _get_nc():
    if "nc" not in _cache:
        _cache["nc"] = _build()
    return _cache["nc"]


def run(inputs, trace=False):
    nc = _get_nc()
    x = np.ascontiguousarray(inputs["x"], dtype=np.float32)
    in_maps = [
        {
            "x": x[c * BL:(c + 1) * BL],
            "weights": np.ascontiguousarray(inputs["weights"], np.float32),
            "RV": np.ascontiguousarray(inputs["RV"], np.float32),
            "alpha": np.ascontiguousarray(inputs["alpha"], np.float32),
        }
        for c in range(NCORES)
    ]
    res = run_bass_kernel_spmd(nc, in_maps, core_ids=list(range(NCORES)),
                               trace=trace)
    out = np.concatenate([r["out"] for r in res.results], axis=0)
    return out, res


def kernel(**inputs) -> np.ndarray:
    out, _ = run(inputs, trace=False)
    return out


# revision 41
# speedup vs baseline: 1.0729x; 1.0161x over previous
"""Binarized conv2d kernel for Trainium2, SPMD over 8 NeuronCores.

Math (forward-value equivalent of the reference):
    real_w  = sum_k RV[k] * weights[k]          # [256,256,3,3], exact fp32 on DVE
    scale   = mean(|real_w|, axis=(1,2,3))      # per out-channel
    out     = conv2d(sign(x), sign(real_w), pad=1) * (scale * alpha)

sign(x) and sign(real_w) are {-1,0,+1} which are exact in fp8e4, so the conv
is computed with fp8 DoubleRow matmuls (exact integer accumulation in fp32
PSUM) and the per-channel scale*alpha is applied on PSUM evacuation.

Sharding: data-parallel over batch, 4 images per core; weights/RV/alpha
replicated. No collectives.
"""

import numpy as np
from contextlib import ExitStack

import concourse.bass as bass
import concourse.bacc as bacc
import concourse.tile as tile
from concourse import mybir
from concourse.bass_utils import run_bass_kernel_spmd
from concourse.masks import make_identity

# Problem shapes (hardcoded per contract)
B, C, H, W = 32, 256, 56, 56
K, KS = 4, 3
NCORES = 8
BL = B // NCORES            # images per core

PW = W + 2                  # padded width 58
PLANE = PW * PW             # 3364
PL = 3376                   # plane stride (>= 1+PLANE+1, multiple of 16)
GO = 1                      # guard offset: plane data starts at elem 1
RPC = 8                     # rows per chunk
CHUNK = RPC * PW            # 464 elems per matmul (one PSUM bank)
NCHUNK = H // RPC           # 7 chunks: psum tile A gets 4, tile B gets 3
PT_CHUNKS = (4, 3)
CIH = C // 128              # 2 ci halves
COH = C // 128              # 2 co halves
TAPS = KS * KS              # 9

F32 = mybir.dt.float32
FP8 = mybir.dt.float8e4
BF16 = mybir.dt.bfloat16

USE_DR = True               # fp8 DoubleRow (2x matmul) vs bf16

_cache = {}


def _build():
    act_dt = FP8 if USE_DR else BF16
    nc = bacc.Bacc("TRN2", target_bir_lowering=False, debug=False,
                   num_devices=NCORES)
    x_d = nc.dram_tensor("x", [BL, C, H, W], F32, kind="ExternalInput")
    w_d = nc.dram_tensor("weights", [K, C, C, KS, KS], F32, kind="ExternalInput")
    rv_d = nc.dram_tensor("RV", [K + 1], F32, kind="ExternalInput")
    al_d = nc.dram_tensor("alpha", [C, 1, 1], F32, kind="ExternalInput")
    o_d = nc.dram_tensor("out", [BL, C, H, W], F32, kind="ExternalOutput")

    with tile.TileContext(nc) as tc, ExitStack() as ctx:
        consts = ctx.enter_context(tc.tile_pool(name="consts", bufs=1))
        wstage = ctx.enter_context(tc.tile_pool(name="wstage", bufs=16))
        wwork = ctx.enter_context(tc.tile_pool(name="wwork", bufs=2))
        xin = ctx.enter_context(tc.tile_pool(name="xin", bufs=2))
        xpads = ctx.enter_context(tc.tile_pool(name="xpads", bufs=1))
        outp = ctx.enter_context(tc.tile_pool(name="outp", bufs=2))

        # --- tiny constant loads on the ACT HWDGE ring (keeps the sync
        # ring free for the big weight DMAs) -------------------------------
        rv = consts.tile([128, K], F32, tag="rv")
        rv_src = bass.AP(tensor=rv_d.ap().tensor, offset=0,
                         ap=[[0, 128], [1, K]])
        nc.scalar.dma_start(out=rv, in_=rv_src)
        alpha_sb = []
        for h in range(COH):
            t = consts.tile([128, 1], F32, tag=f"alpha{h}")
            nc.scalar.dma_start(out=t,
                                in_=al_d.ap()[h * 128:(h + 1) * 128, 0, :])
            alpha_sb.append(t)

        # Padded planes: zero only the pad borders on DVE (tiny strided
        # memsets — the interior is fully overwritten by sign(x) each image
        # and pads are never written again). GpSimd keeps only the identity.
        xpad = []
        for i in range(2):
            t = xpads.tile([128, CIH, PL], act_dt, tag=f"xpad{i}",
                           name=f"xpad{i}")
            for s in range(CIH):
                pl = t[:, s, :]
                # guard + top row + (1,0)
                nc.vector.memset(pl[:, 0:GO + PW + 1], 0.0)
                # (y,0) and (y,57) for y=1..56
                nc.vector.memset(
                    pl[:, GO + PW:GO + PW + H * PW].rearrange(
                        "p (r c) -> p r c", c=PW)[:, :, 0:1], 0.0)
                nc.vector.memset(
                    pl[:, GO + PW + PW - 1:GO + PW + PW - 1 + H * PW].rearrange(
                        "p (r c) -> p r c", c=PW)[:, :, 0:1], 0.0)
                # bottom row + trailing guard/pad
                nc.vector.memset(pl[:, GO + (PW - 1) * PW:PL], 0.0)
            xpad.append(t)
        ident = consts.tile([128, 128], act_dt, tag="ident")
        make_identity(nc, ident)

        wT = consts.tile([128, TAPS, COH, CIH, 128], act_dt, tag="wT")
        scale_alpha = [consts.tile([128, 1], F32, tag=f"sa{h}", name=f"sa{h}")
                       for h in range(COH)]

        # --- weight prep for one co-half: DMA, mix, scale, sign ------------
        # ci-split (HCI columns at a time) so the mix/sign tail trails the
        # weight DMA by one sub-pass instead of the whole 4.7MB.
        HCI = C // CIH * TAPS  # 1152 columns per ci-half
        def prep_half(h):
            # (TensorScalarPtr is DVE-only in walrus codegen — Pool rejects)
            mixeng = nc.vector
            wmix = wwork.tile([128, C * TAPS], F32, tag="wmix", name="wmix")
            ws = wwork.tile([128, C * TAPS], act_dt, tag=f"wsign{h}", bufs=1,
                            name=f"wsign{h}")
            wks = []
            for ci in range(CIH):
                for k in range(K):
                    wk = wstage.tile([128, HCI], F32, tag="wsb", name="wk")
                    wks.append(wk)
                    nc.sync.dma_start(
                        out=wk,
                        in_=w_d.ap()[k, h * 128:(h + 1) * 128,
                                     ci * (C // CIH):(ci + 1) * (C // CIH)]
                        .rearrange("p c a b -> p (c a b)"))
                    dst = wmix[:, ci * HCI:(ci + 1) * HCI]
                    mixeng.scalar_tensor_tensor(
                        dst, wk, rv[:, k:k + 1], wk if k == 0 else dst,
                        mybir.AluOpType.mult,
                        mybir.AluOpType.bypass if k == 0 else
                        mybir.AluOpType.add)
                nc.scalar.sign(ws[:, ci * HCI:(ci + 1) * HCI],
                               wmix[:, ci * HCI:(ci + 1) * HCI])
            return ws, wmix, wks

        # |real_w| row-sums + scale*alpha combine, on DVE (fills DMA-gated
        # bubbles between mix passes)
        def reduce_half(h, wmix):
            absum = consts.tile([128, 1], F32, tag=f"ab{h}", name=f"ab{h}")
            nc.vector.tensor_reduce(absum, wmix, mybir.AxisListType.X,
                                    mybir.AluOpType.add,
                                    apply_absolute_value=True)
            nc.vector.scalar_tensor_tensor(
                scale_alpha[h], absum, 1.0 / (C * TAPS), alpha_sb[h],
                mybir.AluOpType.mult, mybir.AluOpType.mult)

        # --- transpose one co-half's sign-weights into wT -------------------
        # The 18 [128,128] transposes are staged across three PSUM regions
        # (psB: taps 0-5, tps: taps 6-7, psA: tap 8) so the matmuls run
        # back-to-back with only 3 ACT copies and no copy-WAR stalls.
        def transpose_half(h, wsgn, cpsum):
            wsv = wsgn.rearrange("p (ci t) -> p ci t", t=TAPS)
            if h == 0:
                # ps0 first (frees conv00's first psum tile early); psB copy
                # split in two so conv00's tap-0 weights land sooner
                stages = [("ps0", 4 * 512, 8, 9, 1), ("ps1", 3 * 512, 0, 6, 2),
                          ("tps", 512, 6, 8, 1)]
            else:
                # ps1 last: it must wait for the previous conv's ptile-B
                # evacuation, which lands latest
                stages = [("ps0", 4 * 512, 0, 2, 1), ("tps", 512, 2, 4, 1),
                          ("ps1", 3 * 512, 4, 9, 2)]
            for tag, width, ta, tb, ncopy in stages:
                tp = cpsum.tile([128, width], F32, tag=tag, bufs=1,
                                name=f"t{tag}")
                for i, (tap, ci) in enumerate(
                        [(t, c) for t in range(ta, tb) for c in range(CIH)]):
                    nc.tensor.matmul(
                        tp[:, i * 128:(i + 1) * 128],
                        wsv[:, ci * 128:(ci + 1) * 128, tap], ident,
                        start=True, stop=True)
                nt = tb - ta
                for ic in range(ncopy):
                    ca = ta + ic * nt // ncopy
                    cb = ta + (ic + 1) * nt // ncopy
                    o0 = (ca - ta) * CIH * 128
                    nc.scalar.copy(
                        wT[:, ca:cb, h, :, :],
                        tp[:, o0:o0 + (cb - ca) * CIH * 128].rearrange(
                            "p (t ci co) -> p t ci co", t=cb - ca, co=128))

        # --- load + sign one image into its padded plane --------------------
        # DMA rides the sync ring (explicit bandwidth ordering vs weights);
        # the ACT sign is emitted separately so ACT priority is controlled.
        def load(b):
            tiles = []
            for s in range(CIH):
                xs = xin.tile([128, H * W], F32, tag="xsb", name="xsb")
                nc.sync.dma_start(
                    out=xs, in_=x_d.ap()[b, s * 128:(s + 1) * 128].rearrange(
                        "p a b -> p (a b)"))
                tiles.append(xs)
            return tiles

        def sign(b, tiles):
            xp = xpad[b % 2]
            for s in range(CIH):
                dst = xp[:, s, GO:GO + PLANE].rearrange(
                    "p (y x) -> p y x", x=PW)[:, 1:57, 1:57]
                nc.scalar.sign(dst, tiles[s].rearrange("p (y x) -> p y x", x=W))

        # --- conv for one (image, co-half) ---------------------------------
        def conv(b, h, cpsum):
            xp = xpad[b % 2]
            osb = outp.tile([128, H * W], F32, tag="osb", name="osb")
            # consume taps in the order the transpose stages produce them
            # (h=0 stages tap 8 first, h=1 is naturally ordered)
            tap_order = [8, 0, 1, 2, 3, 4, 5, 6, 7] if h == 0 else list(range(TAPS))
            c0 = 0
            for t, nch in enumerate(PT_CHUNKS):
                ps = cpsum.tile([128, nch * 512], F32, tag=f"ps{t}", bufs=1,
                                name=f"ps{t}")
                for itap, tap in enumerate(tap_order):
                    dy, dx = tap // KS - 1, tap % KS - 1
                    lhsT = wT[:, tap, h, :, :]
                    for j in range(nch):
                        c = c0 + j
                        off = GO + (1 + RPC * c + dy) * PW + dx
                        o = ps[:, j * 512:j * 512 + CHUNK]
                        if USE_DR:
                            nc.tensor.matmul(
                                o, lhsT, xp[:, :, off:off + CHUNK],
                                start=(itap == 0), stop=(itap == TAPS - 1),
                                perf_mode=mybir.MatmulPerfMode.DoubleRow)
                        else:
                            for s in range(CIH):
                                nc.tensor.matmul(
                                    o, wT[:, tap, h, s, :],
                                    xp[:, s, off:off + CHUNK],
                                    start=(itap == 0 and s == 0),
                                    stop=(itap == TAPS - 1 and s == CIH - 1))
                src = ps.rearrange("p (c e) -> p c e", e=512)[
                    :, 0:nch, 0:CHUNK].rearrange(
                    "p c (r x) -> p c r x", x=PW)[:, :, :, 1:57]
                dst = osb.rearrange("p (y x) -> p y x", x=W)[
                    :, c0 * RPC:(c0 + nch) * RPC, :].rearrange(
                    "p (c r) x -> p c r x", r=RPC)
                # all PSUM evacuation on ACT (moving any of it to DVE lets
                # the static scheduler interleave it with the weight mix,
                # which measurably regresses)
                nc.scalar.activation(dst, src,
                                     mybir.ActivationFunctionType.Copy,
                                     bias=0.0, scale=scale_alpha[h])
                # per-ptile output DMA on the ACT ring (keeps the sync ring
                # free for input DMAs and avoids head-of-line blocking)
                nc.scalar.dma_start(
                    out=o_d.ap()[b, h * 128:(h + 1) * 128,
                                 c0 * RPC:(c0 + nch) * RPC, :].rearrange(
                        "p a b -> p (a b)"),
                    in_=osb[:, c0 * RPC * W:(c0 + nch) * RPC * W])
                c0 += nch

        # --- schedule --------------------------------------------------------
        # x DMAs ride the ACT ring (issue immediately); sign-x sits after
        # sign-w in program order so the weight chain keeps ACT priority.
        with tc.tile_pool(name="cpsum", bufs=1, space="PSUM") as cpsum:
            xt0 = load(0)              # sync-ring order: x0, w-h0, w-h1, x1..
            sign(0, xt0)
            ws0, wm0, wks0 = prep_half(0)
            # HAM warmup: three fp32 matmuls (~4.6us of PE busy) gated on a
            # mid-stream weight tile so the clock gate opens before the real
            # transposes/convs arrive. Results land in tps and are discarded.
            for i in range(3):
                wtp = cpsum.tile([128, 512], F32, tag="tps", bufs=1,
                                 name="warm")
                nc.tensor.matmul(wtp[:, 0:464], wks0[5][:, 0:128],
                                 wks0[5][:, 0:464], start=True, stop=True)
            transpose_half(0, ws0, cpsum)
            reduce_half(0, wm0)
            ws1, wm1, _ = prep_half(1)
            conv(0, 0, cpsum)
            transpose_half(1, ws1, cpsum)
            reduce_half(1, wm1)
            xt1 = load(1)
            sign(1, xt1)
            conv(0, 1, cpsum)
            for b in range(1, BL):
                if b + 1 < BL:
                    xt = load(b + 1)   # prefetch ahead of this image's evacs
                    sign(b + 1, xt)
                conv(b, 0, cpsum)
                conv(b, 1, cpsum)
    nc.compile()
    return nc


def _get_nc():
    if "nc" not in _cache:
        _cache["nc"] = _build()
    return _cache["nc"]


def run(inputs, trace=False):
    nc = _get_nc()
    x = np.ascontiguousarray(inputs["x"], dtype=np.float32)
    in_maps = [
        {
            "x": x[c * BL:(c + 1) * BL],
            "weights": np.ascontiguousarray(inputs["weights"], np.float32),
            "RV": np.ascontiguousarray(inputs["RV"], np.float32),
            "alpha": np.ascontiguousarray(inputs["alpha"], np.float32),
        }
        for c in range(NCORES)
    ]
    res = run_bass_kernel_spmd(nc, in_maps, core_ids=list(range(NCORES)),
                               trace=trace)
    out = np.concatenate([r["out"] for r in res.results], axis=0)
    return out, res


def kernel(**inputs) -> np.ndarray:
    out, _ = run(inputs, trace=False)
    return out
